# revision 10
# baseline (speedup 1.0000x reference)
"""Trainium2 Bass kernel for nn_DfOpCoefLoop (deep-filter complex FIR + alpha blend).

Reference semantics (per batch b, time t, freq bin f < 96):
    spec_f[t,f] = sum_{i=0..4} x[t+i-2, f] * coefs[t,i,f]      (complex MAC, zero-padded in t)
    out[t,f]    = alpha[t] * spec_f[t,f] + (1-alpha[t]) * x[t,f]
    out[t,f]    = spec[t,f]                                    (f >= 96 passthrough)

Strategy: pure data-parallel over batch (32 batches -> 8 cores x 4 batches).

The host pre-arranges every input in the exact order the engines consume it, so
the device program is 14 large DMAs plus contiguous compute ops:
  - X5: the 5 time-shifted windows, deinterleaved and stacked per t-row as
    [xr taps (5,96) | xi taps (5,96)], zero-padded at the t edges, then blocked
    to (partition = t%128, chunk = t//128) -> one 3.9 MB DMA per batch.
  - CX: coefs per t-row as [cr (5,96) | -ci (5,96)] (ci pre-negated), blocked
    the same way.
  - alpha / (1-alpha) as per-partition scalar tables [128, b*chunk].
Per 128-row time chunk (processed in chunk-pairs to amortize op overhead):
    m1  = X5 * CX = [xr*cr | xi*(-ci)]  --reduce(10 taps)--> re
    m2a = xi*cr, m2b = xr*(-ci)         --reduce, reduce(negated)--> im
    out = alpha * (re|im) + (1-alpha) * x0     (scalar_tensor_tensor + ACT)
m1 alternates DVE/GPSIMD per pair to balance the two elementwise engines; the
reduces run on DVE (only engine with free-dim reduce), (1-alpha)*x0 on ScalarE.
The f>=96 bins never touch the device: the host copies them straight from the
input when assembling the full output (identity passthrough).
"""

import numpy as np

ORDER = 5
LOOKAHEAD = 2
F = 96            # deep-filtered bins
FC = 2 * F        # one t-row of (c,f) planar data: 192 floats
W = ORDER * FC    # 960: stacked taps [xr5 | xi5] / coef row [cr5 | mci5]
NFREQ = 481
B, T = 32, 1000
NCORES = 8
BPC = B // NCORES  # batches per core
PAIR = 2           # chunks per compute op group

_CACHE = {}


def _build_program(bpc, t_len, m1_dve_mask=(1, 0)):
    """Build the per-core Bass program (returns a compiled Bacc)."""
    import concourse.bacc as bacc
    import concourse.mybir as mybir
    import concourse.tile as tile

    nk = (t_len + 127) // 128          # time chunks per batch
    assert nk % PAIR == 0
    ncols = bpc * nk                   # alpha table columns

    # Bacc (not raw Bass): its compile() runs generate_event_semaphores,
    # which splits multi-wait sync onto EventSemaphore instructions --
    # TRN2 instructions encode at most one sem wait.
    nc = bacc.Bacc("TRN2", target_bir_lowering=False, debug=False)
    dt = mybir.dt.float32

    x5_t = nc.dram_tensor("x5_t", [bpc, 128, nk * W], dt, kind="ExternalInput").ap()
    cx_t = nc.dram_tensor("cx_t", [bpc, 128, nk * W], dt, kind="ExternalInput").ap()
    alpha_t = nc.dram_tensor("alpha_t", [128, ncols], dt, kind="ExternalInput").ap()
    oma_t = nc.dram_tensor("oma_t", [128, ncols], dt, kind="ExternalInput").ap()
    outb = nc.dram_tensor("outb", [bpc, 128, nk * FC], dt, kind="ExternalOutput").ap()

    mul = mybir.AluOpType.mult
    add = mybir.AluOpType.add
    copy_fn = mybir.ActivationFunctionType.Copy
    PW = PAIR * W       # free-dim span of one chunk-pair
    HB = ORDER * F      # 480: one comp block of taps

    with tile.TileContext(nc) as tc:
        with (
            tc.tile_pool(name="const", bufs=1) as const_pool,
            tc.tile_pool(name="x5b", bufs=2) as x5_pool,
            tc.tile_pool(name="cxb", bufs=2) as cx_pool,
            tc.tile_pool(name="obp", bufs=2) as ob_pool,
            tc.tile_pool(name="prod", bufs=2) as prod_pool,
            tc.tile_pool(name="small", bufs=3) as small_pool,
        ):
            alpha_sb = const_pool.tile([128, ncols], dt, name="alpha_sb")
            oma_sb = const_pool.tile([128, ncols], dt, name="oma_sb")
            nc.sync.dma_start(alpha_sb[:], alpha_t[:])
            nc.sync.dma_start(oma_sb[:], oma_t[:])

            for b in range(bpc):
                x5b = x5_pool.tile([128, nk * W], dt, name="x5b")
                cxb = cx_pool.tile([128, nk * W], dt, name="cxb")
                ob = ob_pool.tile([128, nk * FC], dt, name="ob")
                nc.sync.dma_start(x5b[:], x5_t[b])
                nc.scalar.dma_start(cxb[:], cx_t[b])

                for q in range(nk // PAIR):
                    qs = q * PW         # start of this pair in x5b/cxb
                    x5q = x5b[:, qs : qs + PW]
                    cxq = cxb[:, qs : qs + PW]

                    p1 = prod_pool.tile([128, PW], dt, name="p1")
                    p2 = prod_pool.tile([128, PW], dt, name="p2")
                    acc = small_pool.tile([128, PAIR * FC], dt, name="acc")
                    sa = small_pool.tile([128, PAIR * F], dt, name="sa")
                    sb_t = small_pool.tile([128, PAIR * F], dt, name="sb_t")
                    v = small_pool.tile([128, PAIR * FC], dt, name="v")

                    # m1 = X5 * CX (fully contiguous); alternate engine
                    m1_eng = nc.vector if m1_dve_mask[(b + q) % len(m1_dve_mask)] else nc.gpsimd
                    m1_eng.tensor_mul(p1[:], x5q, cxq)
                    # re = sum over the 10 (comp,tap) products
                    nc.vector.tensor_reduce(
                        acc[:].rearrange("p (j c f) -> p j c f", j=PAIR, c=2, f=F)[
                            :, :, 0:1
                        ].squeeze(2),
                        p1[:].rearrange("p (j gi f) -> p j f gi", j=PAIR, gi=2 * ORDER, f=F),
                        axis=mybir.AxisListType.X,
                        op=add,
                    )
                    # m2a = xi*cr ; m2b = xr*(-ci)
                    nc.gpsimd.tensor_mul(
                        p2[:, 0 : PAIR * HB].rearrange("p (j e) -> p j e", j=PAIR),
                        x5q.rearrange("p (j g e) -> p j g e", j=PAIR, g=2, e=HB)[
                            :, :, 1:2
                        ].squeeze(2),
                        cxq.rearrange("p (j g e) -> p j g e", j=PAIR, g=2, e=HB)[
                            :, :, 0:1
                        ].squeeze(2),
                    )
                    nc.gpsimd.tensor_mul(
                        p2[:, PAIR * HB : 2 * PAIR * HB].rearrange(
                            "p (j e) -> p j e", j=PAIR
                        ),
                        x5q.rearrange("p (j g e) -> p j g e", j=PAIR, g=2, e=HB)[
                            :, :, 0:1
                        ].squeeze(2),
                        cxq.rearrange("p (j g e) -> p j g e", j=PAIR, g=2, e=HB)[
                            :, :, 1:2
                        ].squeeze(2),
                    )
                    nc.vector.tensor_reduce(
                        sa[:].rearrange("p (j f) -> p j f", j=PAIR),
                        p2[:, 0 : PAIR * HB].rearrange(
                            "p (j i f) -> p j f i", j=PAIR, i=ORDER, f=F
                        ),
                        axis=mybir.AxisListType.X,
                        op=add,
                    )
                    nc.vector.tensor_reduce(
                        sb_t[:].rearrange("p (j f) -> p j f", j=PAIR),
                        p2[:, PAIR * HB : 2 * PAIR * HB].rearrange(
                            "p (j i f) -> p j f i", j=PAIR, i=ORDER, f=F
                        ),
                        axis=mybir.AxisListType.X,
                        op=add,
                        negate=True,
                    )
                    # im = sa + sb
                    nc.vector.tensor_add(
                        acc[:].rearrange("p (j c f) -> p j c f", j=PAIR, c=2, f=F)[
                            :, :, 1:2
                        ].squeeze(2),
                        sa[:].rearrange("p (j f) -> p j f", j=PAIR),
                        sb_t[:].rearrange("p (j f) -> p j f", j=PAIR),
                    )
                    # blend per chunk (alpha is a per-(b,chunk) partition scalar)
                    for kk in range(PAIR):
                        col = b * nk + q * PAIR + kk
                        ks = (q * PAIR + kk) * W
                        # v = (1-alpha) * x0 ; x0 = tap d=0 of X5 (planar view)
                        nc.scalar.activation(
                            v[:, kk * FC : (kk + 1) * FC].rearrange(
                                "p (c f) -> p c f", c=2, f=F
                            ),
                            bass_view_x0(x5b, ks),
                            copy_fn,
                            scale=oma_sb[:, col : col + 1],
                        )
                        # out = alpha*acc + v
                        nc.vector.scalar_tensor_tensor(
                            ob[:, (q * PAIR + kk) * FC : (q * PAIR + kk + 1) * FC],
                            acc[:, kk * FC : (kk + 1) * FC],
                            alpha_sb[:, col : col + 1],
                            v[:, kk * FC : (kk + 1) * FC],
                            op0=mul,
                            op1=add,
                        )

                nc.sync.dma_start(outb[b], ob[:])
    nc.compile()
    return nc


def bass_view_x0(x5b, ks):
    """View of tap d=0 (re then im plane) within one chunk column of x5b."""
    # chunk row layout: [xr d0..d4 (5*96) | xi d0..d4]; d=0 is i=LOOKAHEAD
    return x5b[:, ks : ks + W].rearrange(
        "p (c i f) -> p c i f", c=2, i=ORDER, f=F
    )[:, :, LOOKAHEAD : LOOKAHEAD + 1].squeeze(2)


def _get_program(bpc=BPC, t_len=T):
    key = (bpc, t_len)
    if key not in _CACHE:
        _CACHE[key] = _build_program(bpc, t_len)
    return _CACHE[key]


def _block(a, nk):
    """[t_pad*128... (nk*128, R)] -> [128, nk*R] with partition = t%128."""
    n, r = a.shape
    assert n == nk * 128
    return np.ascontiguousarray(
        a.reshape(nk, 128, r).transpose(1, 0, 2).reshape(128, nk * r)
    )


def _host_prep(spec, coefs, alpha, bpc, t_len):
    """Re-layout one core's inputs into the device consumption order."""
    nk = (t_len + 127) // 128
    tp = nk * 128
    spec2 = np.asarray(spec[:, 0], dtype=np.float32)          # (bpc, t, 481, 2)
    xr = spec2[:, :, :F, 0]                                    # (bpc, t, 96)
    xi = spec2[:, :, :F, 1]
    xrp = np.zeros((bpc, tp + ORDER - 1, F), np.float32)
    xip = np.zeros((bpc, tp + ORDER - 1, F), np.float32)
    xrp[:, LOOKAHEAD : LOOKAHEAD + t_len] = xr
    xip[:, LOOKAHEAD : LOOKAHEAD + t_len] = xi
    # taps: X5[t, i, f] = x[t + i - LOOKAHEAD]
    xr5 = np.stack([xrp[:, i : i + tp] for i in range(ORDER)], axis=2)  # (bpc,tp,5,96)
    xi5 = np.stack([xip[:, i : i + tp] for i in range(ORDER)], axis=2)
    x5 = np.concatenate(
        [xr5.reshape(bpc, tp, ORDER * F), xi5.reshape(bpc, tp, ORDER * F)], axis=2
    )                                                          # (bpc, tp, 960)

    cr = np.asarray(coefs[..., 0], dtype=np.float32).reshape(bpc, t_len, ORDER * F)
    ci = np.asarray(coefs[..., 1], dtype=np.float32).reshape(bpc, t_len, ORDER * F)
    cx = np.zeros((bpc, tp, W), np.float32)
    cx[:, :t_len, : ORDER * F] = cr
    cx[:, :t_len, ORDER * F :] = -ci

    x5_t = np.stack([_block(x5[b], nk) for b in range(bpc)])
    cx_t = np.stack([_block(cx[b], nk) for b in range(bpc)])

    al = np.zeros((bpc, tp), np.float32)
    al[:, :t_len] = alpha[:, :, 0]
    alpha_t = np.ascontiguousarray(
        al.reshape(bpc, nk, 128).transpose(2, 0, 1).reshape(128, bpc * nk)
    )
    oma_t = np.ascontiguousarray(1.0 - alpha_t)
    return {
        "x5_t": x5_t,
        "cx_t": cx_t,
        "alpha_t": alpha_t,
        "oma_t": oma_t,
    }


def _unblock_out(ob, t_len):
    """[128, nk*192] planar (c,f) blocked -> (t, 96, 2) interleaved."""
    nk = ob.shape[1] // FC
    a = ob.reshape(128, nk, 2, F).transpose(1, 0, 2, 3).reshape(nk * 128, 2, F)
    return np.ascontiguousarray(a[:t_len].transpose(0, 2, 1))  # (t, 96, 2)


def run_on_cores(spec, coefs, alpha, trace=False):
    """Full-input entry: shard, run on 8 cores, return (out_full, results_obj)."""
    from concourse import bass_utils

    nc = _get_program()
    in_maps = [
        _host_prep(
            spec[c * BPC : (c + 1) * BPC],
            coefs[c * BPC : (c + 1) * BPC],
            alpha[c * BPC : (c + 1) * BPC],
            BPC,
            T,
        )
        for c in range(NCORES)
    ]
    res = bass_utils.run_bass_kernel_spmd(
        nc, in_maps, core_ids=list(range(NCORES)), trace=trace
    )
    full = np.array(spec, dtype=np.float32, copy=True)  # f>=96 passthrough on host
    for c in range(NCORES):
        ob = res.results[c]["outb"]
        for b in range(BPC):
            full[c * BPC + b, 0, :, :F, :] = _unblock_out(ob[b], T)
    return full, res


def kernel(spec, coefs, alpha):
    spec = np.asarray(spec, dtype=np.float32)
    coefs = np.asarray(coefs, dtype=np.float32)
    alpha = np.asarray(alpha, dtype=np.float32)
    full, _ = run_on_cores(spec, coefs, alpha, trace=False)
    return full
